# revision 45
# baseline (speedup 1.0000x reference)
"""VQ codebook kernel v6 (nn_ApplyKmeans): fp8 DoubleRow matmul + fused
packed-argmax; index decode + codeword gather moved to host.

Per core (data-parallel over rows of x, 8 cores):
  - TensorE: psum = (32x)_fp8e4 @ (32C)_fp8e4 via DoubleRow (256-deep
    contraction per pass, 2x fp16 throughput) -> psum = 1024*(x.C) + noise.
    Dummy warmup matmuls bridge the DMA-ring bring-up (p-state climb).
  - DVE custom op ARGMAX_PACK32 (one full pass over psum):
        a    = psum + (MAGIC32 - cnb1024)   rounds to a multiple of 32
        q32  = a - MAGIC32                  = 32*round(32*score)
        P_k  = q32 + (1023.5 - k)/32        exact fp32 pack of (score, index)
        out  = P streamed to SBUF; accum_out = max_k P  -> packed argmax
    cnb (the -Cnorm/2 bias, pre-shifted by MAGIC32) is broadcast to all
    partitions on-device via a 3-row bf16 matmul against ones.
  - Pool: negT = theta_P - P* (per 128-row subtile, no cross-subtile barrier)
  - ScalarE: rs = sum relu(P + negT): flags rows with a runner-up within
    theta of the max (fp8 noise margin).
  - Device outputs ONLY pk/rs ([128, 256] fp32 each). v4 gathered fp16
    codewords on-device; that pushed per-core DMA to ~345 GB/s (the 358
    GB/s ceiling) and stalled the matmul stream about once per tile.
  - Host: decode k* from the packed P* (exact fp32 pack), gather fp32
    codewords C.T[k*], exact fp32 rescore of flagged rows.

v4 (on-device gather) measured 413.7 us; v6 measures ~352-354 us with the
PE matmul stream >99% packed at the DVFS-throttled fp8 rate.
"""

import sys

sys.path.insert(0, "/opt/trn_rl_repo")

import numpy as np
import ml_dtypes

import concourse.bass as bass  # noqa: F401  (kept: bass types referenced via bacc)
import concourse.mybir as mybir
from concourse import bacc
from concourse.tile import TileContext
from concourse.bass_utils import run_bass_kernel_spmd

N, D, K = 262144, 768, 1024
NCORES = 8
NSH = N // NCORES            # 32768 rows per core
DCH = D // 128               # 6 contraction chunks
MT = 512                     # rows per DMA tile
NOT = NSH // MT              # 64 outer tiles
NST = NSH // 128             # 256 sub-tiles of 128 rows

MAGIC = 12582912.0           # 1.5 * 2^23: fp32 round-to-int magic constant
MAGIC32 = 402653184.0        # 3 * 2^27: rounds fp32 to a multiple of 32
THETA32 = 176.0              # flag threshold in 1/32-raw score units
THETA_P = THETA32 * 32.0     # threshold in packed-P units (P = 32*Q + idx/32)
FLAG_SLACK = 32.0 + 8.0      # index wobble + fp32 accum slop
PK_OVERFLOW = 8000.0 * 32.0  # |P*| above this risks losing index bits: flag

FP8 = ml_dtypes.float8_e4m3

# ---------------------------------------------------------------- custom DVE ops


def _ref_argmax_pack(in0, in1, s0, s1, imm2):
    # in1 = (MAGIC32 - cnb1024): the add rounds (psum - cnb1024) to a
    # multiple of 32; subtracting MAGIC32 leaves 32*Q exactly. The scan
    # contributes C1 - (k+1)*imm2 = (1023.5 - k)/32 for C1 = 32.015625.
    p = in0.astype(np.float32).reshape(in0.shape[0], -1)
    mcnb = np.asarray(in1, np.float32).reshape(p.shape[0], -1)
    m = np.float32(np.asarray(s0).reshape(-1)[0] if isinstance(s0, np.ndarray) else s0)
    c1 = np.float32(np.asarray(s1).reshape(-1)[0] if isinstance(s1, np.ndarray) else s1)
    step = np.float32(imm2)
    a = (p + mcnb).astype(np.float32)
    q32 = (a - m).astype(np.float32)
    iota = np.arange(p.shape[1], dtype=np.float32)
    pk = (q32 + (c1 - step * (iota + np.float32(1.0)))[None, :]).astype(np.float32)
    acc = pk.max(axis=1, keepdims=True)
    return pk, acc


def _ref_idx_extract(in0, in1, s0, s1, imm2):
    p = in0.astype(np.float32).reshape(in0.shape[0], -1)
    c3 = np.asarray(in1, np.float32).reshape(-1, 1)
    half = np.float32(np.asarray(s0).reshape(-1)[0] if isinstance(s0, np.ndarray) else s0)
    m = np.float32(np.asarray(s1).reshape(-1)[0] if isinstance(s1, np.ndarray) else s1)
    u = (p * np.float32(imm2)).astype(np.float32)
    q = (((u - half) + m) - m).astype(np.float32)
    f = (u - q).astype(np.float32)
    k = ((np.float32(1.0) - f) * c3 - half).astype(np.float32)
    return k


def _make_ops():
    from concourse import dve_ops
    from concourse.dve_ops import DveOp
    from concourse.dve_spec import (
        Spec, Src0, Src1, C0, C1, C2, C3, One, maxx, lower, scan,
        AluOp, _has_src1, _spill_c3_to_src1,
    )
    from concourse.dve_uop import DveOpSpec

    if "ARGMAX_PACK32_ANT9" in dve_ops._SUB_OPCODE_FOR_NAME:
        by_name = {o.name: o for o in dve_ops.OPS}
        return by_name["ARGMAX_PACK32_ANT9"], by_name["IDX_EXTRACT32_ANT9"]

    # Src1 = (MAGIC32 - cnb1024); C0 = MAGIC32. The descending scan steps by
    # imm2 = 1/32, yielding C1 - (k+1)/32 = (1023.5 - k)/32 for C1 = 1024.5/32.
    _q32 = (Src0 + Src1) - C0
    _down = scan(AluOp.SUBTRACT, C2, init=C1)
    argmax_spec = Spec(
        body=_q32 + _down,
        accum=maxx,
        reference=_ref_argmax_pack,
    )
    op_argmax = DveOp("ARGMAX_PACK32_ANT9", argmax_spec, subdim=False, uops_sha={})

    _u = Src0 * C2
    _qq = ((_u - C0) + C1) - C1
    _f = _u - _qq
    idx_spec = Spec(
        body=_spill_c3_to_src1((One - _f) * C3 - C0),
        reference=_ref_idx_extract,
    )
    op_idx = DveOp("IDX_EXTRACT32_ANT9", idx_spec, subdim=False, uops_sha={})

    for op in (op_argmax, op_idx):
        row = max(dve_ops._SUB_OPCODE_FOR_NAME.values()) + 1
        assert row < 0x20
        dve_ops._SUB_OPCODE_FOR_NAME[op.name] = row
        dve_ops.OPS.append(op)
        dve_ops.CUSTOM_DVE_SPECS[op.name] = op.spec
        for ver in ("v3", "v4"):
            try:
                s = DveOpSpec(
                    name=op.name,
                    opcode=row,
                    uops=lower(op.spec, ver=ver),
                    rd1_en=_has_src1(op.spec),
                )
                op.uops_sha[ver] = s.sha(ver)
            except Exception as e:  # noqa: BLE001
                print(f"warn: {op.name} lower({ver}) failed: {e}", file=sys.stderr)
    return op_argmax, op_idx


OP_ARGMAX, OP_IDX = _make_ops()

# ---------------------------------------------------------------------- kernel

# p-state warmup matmuls while the first DMAs land. N=512 at the low
# p-state is ~650 ns each; 11 of them bridge memset-done (~7.6 us) to
# the first real matmul's data arrival (9.9-13 us across runs, DMA
# ring bring-up jitter) with the PE p-state warm the whole way.
N_WARMUP_MM = 11


def emit(nc, xt, cb, cnb_e, pk_e, rsa_e, n_outer):
    nst = n_outer * (MT // 128)
    nsub = MT // 128
    with TileContext(nc) as tc:
        with (
            tc.tile_pool(name="const", bufs=1) as const_pool,
            tc.tile_pool(name="xp", bufs=6) as xpool,
            tc.tile_pool(name="pst", bufs=8) as pstpool,
            tc.tile_pool(name="ntp", bufs=8) as ntpool,
            tc.tile_pool(name="ps", bufs=4, space="PSUM") as pspool,
        ):
            # Warmup: one memset, then dummy DoubleRow matmuls keep the PE
            # busy (p-state climb) while the csb/x DMAs stream in. lhsT and
            # rhs share the buffer; the psum is never read.
            wmov = const_pool.tile([128, 2, 512], mybir.dt.float8e4)
            nc.gpsimd.memset(wmov[:], 1.0)
            wps = pspool.tile([128, K], mybir.dt.float32, space="PSUM", tag="ps")
            for _ in range(N_WARMUP_MM):
                nc.tensor.matmul(
                    out=wps[:, :512],
                    lhsT=wmov[:, :, :128],
                    rhs=wmov[:],
                    start=True,
                    stop=True,
                    perf_mode=mybir.MatmulPerfMode.DoubleRow,
                )

            # Inputs for the on-device cnb broadcast (see the ot==0/j==0
            # block below): a 6 KiB bf16 [MAGIC32; v_hi; v_lo] table and a
            # ones vector.
            cnbh = const_pool.tile([3, K], mybir.dt.bfloat16)
            nc.sync.dma_start(out=cnbh[:], in_=cnb_e[:])
            ones3 = const_pool.tile([3, 128], mybir.dt.bfloat16)
            nc.gpsimd.memset(ones3[:], 1.0)

            csb = const_pool.tile([128, DCH, K], mybir.dt.float8e4)
            cb_r = cb[:].rearrange("(c p) k -> p c k", p=128)
            cnb = const_pool.tile([128, K], mybir.dt.float32)
            pkbuf = const_pool.tile([128, nst], mybir.dt.float32)
            rsabuf = const_pool.tile([128, nst], mybir.dt.float32)

            xtiles = [None] * n_outer
            # First DMAs in dependency-priority order: codebook chunk 0, the
            # first x subtile, rest of the codebook + first tile. The 512
            # KiB cnb broadcast is deferred past tile 1 (the DVE only needs
            # it ~3 subtiles in; the psum pool hides the wait).
            xtiles[0] = xpool.tile(
                [128, DCH, MT], mybir.dt.float8e4, tag="xt", name="xtile"
            )
            nc.sync.dma_start(out=csb[:, 0:2, :], in_=cb_r[:, 0:2, :])
            nc.sync.dma_start(
                out=xtiles[0][:, :, 0:128], in_=xt[0][:, :, 0:128]
            )
            for cc in range(1, DCH // 2):
                nc.sync.dma_start(
                    out=csb[:, 2 * cc:2 * cc + 2, :], in_=cb_r[:, 2 * cc:2 * cc + 2, :]
                )
            for j in range(1, nsub):
                nc.sync.dma_start(
                    out=xtiles[0][:, :, j * 128:(j + 1) * 128],
                    in_=xt[0][:, :, j * 128:(j + 1) * 128],
                )

            for ot in range(n_outer):
                if ot > 0:
                    xtile = xpool.tile([128, DCH, MT], mybir.dt.float8e4, tag="xt")
                    xtiles[ot] = xtile
                    if ot <= 5:
                        # Finer granularity while the DMA queues ramp up.
                        for j in range(nsub):
                            nc.sync.dma_start(
                                out=xtile[:, :, j * 128:(j + 1) * 128],
                                in_=xt[ot][:, :, j * 128:(j + 1) * 128],
                            )
                    else:
                        nc.sync.dma_start(out=xtile[:], in_=xt[ot])
                xtile = xtiles[ot]
                negTt = ntpool.tile([128, nsub], mybir.dt.float32, tag="nt")
                for j in range(nsub):
                    t = ot * nsub + j
                    psum = pspool.tile([128, K], mybir.dt.float32, space="PSUM", tag="ps")
                    for dp in range(DCH // 2):
                        for h in range(2):
                            nc.tensor.matmul(
                                out=psum[:, h * 512:(h + 1) * 512],
                                lhsT=xtile[:, 2 * dp:2 * dp + 2, j * 128:(j + 1) * 128],
                                rhs=csb[:, 2 * dp:2 * dp + 2, h * 512:(h + 1) * 512],
                                start=(dp == 0),
                                stop=(dp == DCH // 2 - 1),
                                perf_mode=mybir.MatmulPerfMode.DoubleRow,
                            )
                    if ot == 0 and j == 0:
                        # cnb broadcast, on-device: a 3-row bf16 matmul
                        # against ones sums [MAGIC32; v_hi; v_lo] into psum
                        # (exact to one 32-quantum of the packed score,
                        # inside the flag margin), replacing a 512 KiB
                        # host-side broadcast DMA that crowded the early
                        # queue. Slotted after the first real subtile so it
                        # never delays the matmul stream start; the first
                        # ARGMAX (its only consumer) runs right after the
                        # tensor_copy anyway.
                        cpsum = pspool.tile(
                            [128, K], mybir.dt.float32, space="PSUM", tag="ps"
                        )
                        for h in range(2):
                            nc.tensor.matmul(
                                out=cpsum[:, h * 512:(h + 1) * 512],
                                lhsT=ones3[:],
                                rhs=cnbh[:, h * 512:(h + 1) * 512],
                                start=True,
                                stop=True,
                            )
                        nc.vector.tensor_copy(cnb[:], cpsum[:])
                    pstr = pstpool.tile([128, K], mybir.dt.float32, tag="pst")
                    nc.vector._custom_dve(
                        OP_ARGMAX,
                        out=pstr[:],
                        in0=psum[:],
                        in1=cnb[:],
                        s0=MAGIC32,
                        s1=1024.5 / 32.0,
                        imm2=1.0 / 32.0,
                        accum_out=pkbuf[:, t:t + 1],
                    )
                    # Flag bias for this subtile only (Pool engine; v4 computed
                    # it per 4-subtile tile, serializing the relu chain).
                    nc.gpsimd.tensor_scalar(
                        out=negTt[:, j:j + 1],
                        in0=pkbuf[:, t:t + 1],
                        scalar1=-1.0,
                        scalar2=THETA_P,
                        op0=mybir.AluOpType.mult,
                        op1=mybir.AluOpType.add,
                    )
                    # Runner-up flag pass. out is discarded: write in place
                    # over the P stream it reads (reads lead writes in
                    # element order; the region is dead after this
                    # instruction). Only accum_out is used.
                    nc.scalar.activation(
                        out=pstr[:],
                        in_=pstr[:],
                        func=mybir.ActivationFunctionType.Relu,
                        bias=negTt[:, j:j + 1],
                        scale=1.0,
                        accum_out=rsabuf[:, t:t + 1],
                    )
                # Stream the long-retired first half of pk/rs out mid-loop
                # so the final DMA is small. Only columns finished ~16 tiles
                # (~90 us) earlier are touched here — keeping a wide safety
                # margin to the in-flight accum_out writes.
                if ot == 47:
                    nc.sync.dma_start(out=pk_e[:, 0:128], in_=pkbuf[:, 0:128])
                    nc.sync.dma_start(out=rsa_e[:, 0:128], in_=rsabuf[:, 0:128])
            nc.sync.dma_start(out=pk_e[:, 128:nst], in_=pkbuf[:, 128:nst])
            nc.sync.dma_start(out=rsa_e[:, 128:nst], in_=rsabuf[:, 128:nst])


def build_kernel(n_outer=NOT):
    nst = n_outer * (MT // 128)
    nc = bacc.Bacc()
    xt = nc.declare_dram_parameter("xt", [n_outer, 128, DCH, MT], mybir.dt.float8e4, isOutput=False)
    cb = nc.declare_dram_parameter("cb", [D, K], mybir.dt.float8e4, isOutput=False)
    cnb_e = nc.declare_dram_parameter("cnb", [3, K], mybir.dt.bfloat16, isOutput=False)
    pk_e = nc.declare_dram_parameter("pk", [128, nst], mybir.dt.float32, isOutput=True)
    rsa_e = nc.declare_dram_parameter("rsa", [128, nst], mybir.dt.float32, isOutput=True)
    emit(nc, xt, cb, cnb_e, pk_e, rsa_e, n_outer)
    nc.finalize()
    return nc


# ------------------------------------------------------------------- host side


def _prep_core(args):
    x, c = args
    xs = x[c * NSH:(c + 1) * NSH]
    xh = (32.0 * xs).astype(FP8)
    # xprep[ot, p, cch, j, q] = xh[512*ot + 4*q + j, cch*128 + p]
    v = xh.reshape(NOT, 128, 4, DCH, 128)        # [ot, q, j, cch, p]
    v = v.transpose(0, 4, 3, 2, 1)               # [ot, p, cch, j, q]
    return np.ascontiguousarray(v).reshape(NOT, 128, DCH, MT)


def prepare_in_maps(x, C, Cnorm):
    x = np.ascontiguousarray(np.asarray(x, dtype=np.float32))
    C = np.ascontiguousarray(np.asarray(C, dtype=np.float32))
    Cnorm = np.asarray(Cnorm, dtype=np.float32).reshape(1, K)

    from concurrent.futures import ThreadPoolExecutor
    with ThreadPoolExecutor(max_workers=8) as ex:
        xts = list(ex.map(_prep_core, [(x, c) for c in range(NCORES)]))

    cb = (32.0 * C).astype(FP8)
    # cnb as bf16 rows [MAGIC32; v_hi; v_lo]: summed on-device into the
    # [128, K] fp32 broadcast. v_hi is a multiple of 2048 (exact in bf16
    # up to +-500k), |v_lo| <= 1024 (bf16 error <= 4, well under the
    # 32-quantum of the pack).
    v = (1024.0 * (384.0 - 0.5 * Cnorm.astype(np.float64))).reshape(K)
    assert np.abs(v).max() < 500000.0
    v_hi = np.round(v / 2048.0) * 2048.0
    v_lo = v - v_hi
    cnb = np.ascontiguousarray(
        np.stack([np.full(K, MAGIC32), v_hi, v_lo]).astype(ml_dtypes.bfloat16)
    )
    return [{"xt": xts[c], "cb": cb, "cnb": cnb} for c in range(NCORES)]


# row(q, t) = 512*(t//4) + 4*q + (t%4)  (see _prep_core layout)
_Q = np.arange(128)[:, None]
_T = np.arange(NST)[None, :]
_ROWS = (512 * (_T // 4) + 4 * _Q + (_T % 4)).ravel()


def _decode_ids(pk):
    """Exact decode of k* from the packed fp32 P* = 32*Q + (1023.5-k)/32."""
    p = pk.astype(np.float64).ravel()
    q = np.floor(p / 32.0)
    rem = p - 32.0 * q                     # (1023.5-k)/32 in (0, 32)
    k = np.rint(1023.5 - 32.0 * rem).astype(np.int64)
    np.clip(k, 0, K - 1, out=k)            # overflow rows are flagged anyway
    return k


def _post_core(args):
    res, CT, out, c = args
    ids = _decode_ids(res["pk"])           # [128*NST]
    out[c * NSH + _ROWS] = CT[ids]
    flag = (res["rsa"] > (THETA_P + FLAG_SLACK)) | (np.abs(res["pk"]) > PK_OVERFLOW)
    return c * NSH + _ROWS[flag.ravel()]


def postprocess(results, x, C, Cnorm):
    """Decode ids, gather fp32 codewords, exactly rescore flagged rows."""
    x = np.asarray(x, dtype=np.float32)
    C = np.asarray(C, dtype=np.float32)
    Cnorm = np.asarray(Cnorm, dtype=np.float32).reshape(1, K)
    CT = np.ascontiguousarray(C.T)
    out = np.empty((N, D), dtype=np.float32)

    from concurrent.futures import ThreadPoolExecutor
    with ThreadPoolExecutor(max_workers=8) as ex:
        recheck = list(
            ex.map(_post_core, [(results[c], CT, out, c) for c in range(NCORES)])
        )

    rows = np.concatenate(recheck)
    if len(rows):
        # chunked exact fp32 rescore (single-core host: keep peak memory low)
        ids = np.empty(len(rows), dtype=np.int64)
        CS = 16384
        for i in range(0, len(rows), CS):
            xr = x[rows[i:i + CS]]
            dist = (
                np.sum(xr * xr, axis=1, keepdims=True)
                - 2.0 * (xr @ C)
                + Cnorm
            )
            ids[i:i + CS] = np.argmin(dist, axis=1)
        out[rows] = CT[ids]
    return out


def kernel(x, C, Cnorm):
    in_maps = prepare_in_maps(x, C, Cnorm)
    nc = build_kernel()
    res = run_bass_kernel_spmd(nc, in_maps, core_ids=list(range(NCORES))).results
    return postprocess(res, x, C, Cnorm)
